# revision 34
# baseline (speedup 1.0000x reference)
"""Trainium2 Bass kernel for nn_Attention_53231824666818 (GQA attention block).

Sharding: tensor-parallel over heads across 8 NeuronCores. Core c owns query
heads {2c, 2c+1} and kv head c//4 (kv-head groups stay aligned to cores).
Each core computes a full-shape partial of the output projection (row-sharded
Wo); the host sums the 8 partials.

v2 redesign vs the first working kernel:
  * all operands staged in fp16 (halves HBM traffic, enables FWL weight
    loads and DVE 2x modes); PSUM accumulation stays fp32.
  * both query heads fused per tile: scores/exp/PV/softmax-denominator all
    operate on [128, 1024] tiles (head0 | head1), halving op counts.
  * every reciprocal/rsqrt computed as exp(-ln(x)) on the scalar engine --
    the DVE iterative-divide (3.3us per [128,512]) is gone, and ln/exp/copy/
    square all live in ONE activation table set (no table reloads).
  * causal mask applied additively on the score PSUM before a single full
    width exp (no subrange exp calls).
  * attention inner loop software-pipelined by one k-tile so the PE never
    waits on the exp.
  * weights/tables loaded in a handful of ~1MB DMAs; per-strip activations
    in one 2MB DMA; output staged fp16.
"""

import math

import numpy as np

import concourse.bass as bass
import concourse.tile as tile
from concourse import mybir

# ---------------------------------------------------------------------------
# Problem constants (hardcoded; kernel.py must be self-contained).
B, S, HID = 1, 2048, 2048
NH, NKV, HD = 16, 2, 128
EPS = 1e-6
THETA = 1000000.0
NCORES = 8
HPC = NH // NCORES          # query heads per core (2)
SW = 512                    # seq strip width
NSTRIP = S // SW            # 4
NHT = HID // 128            # hid-dim k-tiles (16)
ISQ = 1.0 / math.sqrt(HD)
MASKNEG = -60000.0

F32 = mybir.dt.float32
F16 = mybir.dt.float16

_ALU = mybir.AluOpType
_ACT = mybir.ActivationFunctionType


# ---------------------------------------------------------------------------
# Wait legalization: this walrus build caps fused sync waits at 1 per
# instruction (2 for event-semaphore ops) and rejects any wait on the
# LDWEIGHTS half of a lowered matmul. Tile can attach several waits to one
# instruction (notably the kernel-tail drain), so after TileContext exit we
# hoist excess waits onto same-engine InstNoOp's placed immediately before
# the owner, which blocks the sequencer identically.
_LW_COUNTER = [0]


def _wait_cap(ins) -> int:
    nm = type(ins).__name__
    if nm == "InstMatmult":
        return 0
    if "EventSem" in nm:
        return 2
    return 1


def legalize_waits(nc):
    for fn in nc.m.functions:
        for bb in fn.blocks:
            out = []
            changed = False
            for ins in bb.instructions:
                si = ins.sync_info
                if si is not None:
                    waits = list(si.on_wait or [])
                    cap = _wait_cap(ins)
                    if len(waits) > cap:
                        changed = True
                        for w in waits[cap:]:
                            _LW_COUNTER[0] += 1
                            nop = mybir.InstNoOp(
                                name=f"I-lw-{_LW_COUNTER[0]}",
                                engine=ins.engine,
                                sync_info=mybir.SyncInfo(on_wait=[w], on_update=[]),
                            )
                            out.append(nop)
                        ins.sync_info = mybir.SyncInfo(
                            on_wait=waits[:cap], on_update=list(si.on_update or [])
                        )
                out.append(ins)
            if changed:
                bb.instructions = out
    return nc


# ---------------------------------------------------------------------------
PHASE_MARKS = []


def _mark(nc, label):
    PHASE_MARKS.append((label, int(nc.get_next_instruction_name().split("-")[1])))


def build_nc(legalize=True):
    PHASE_MARKS.clear()
    nc = bass.Bass()

    # host-pretiled activations: column (s*NHT + ht)*SW + c holds
    # X[512 s + c, 128 ht + p] -- every DMA chunk is contiguous per partition
    xT = nc.dram_tensor("xT", [128, NSTRIP * NHT * SW], F16,
                        kind="ExternalInput")
    wq = nc.dram_tensor("wq", [128, NHT * HPC * HD], F16, kind="ExternalInput")
    wk = nc.dram_tensor("wk", [128, NHT * HD], F16, kind="ExternalInput")
    wv = nc.dram_tensor("wv", [128, NHT * HD], F16, kind="ExternalInput")
    wo = nc.dram_tensor("wo", [128, HPC * HID], F16, kind="ExternalInput")
    cosq = nc.dram_tensor("cosq", [HD, NSTRIP * 2 * SW], F16, kind="ExternalInput")
    sinq = nc.dram_tensor("sinq", [HD, NSTRIP * 2 * SW], F16, kind="ExternalInput")
    cosk = nc.dram_tensor("cosk", [HD, S], F16, kind="ExternalInput")
    sink = nc.dram_tensor("sink", [HD, S], F16, kind="ExternalInput")
    wqn = nc.dram_tensor("wqn", [HD, 1], F32, kind="ExternalInput")
    wkn = nc.dram_tensor("wkn", [HD, 1], F32, kind="ExternalInput")
    tmask = nc.dram_tensor("tmask", [128, 128], F16, kind="ExternalInput")
    selm = nc.dram_tensor("selm", [128, 4 * SW], F16, kind="ExternalInput")
    onesm = nc.dram_tensor("onesm", [128, 128], F16, kind="ExternalInput")
    ident = nc.dram_tensor("ident", [128, 128], F16, kind="ExternalInput")
    epsb = nc.dram_tensor("epsb", [HD, 1], F32, kind="ExternalInput")
    out = nc.dram_tensor("out", [S, HID], F16, kind="ExternalOutput")

    with tile.TileContext(nc) as tc:
        with tc.tile_pool(name="persist", bufs=1) as pp, \
             tc.tile_pool(name="xtp", bufs=2) as xp, \
             tc.tile_pool(name="epi", bufs=2) as ep, \
             tc.tile_pool(name="exp", bufs=3) as xep, \
             tc.tile_pool(name="obp", bufs=2) as obp, \
             tc.tile_pool(name="psA", bufs=2, space="PSUM") as pA, \
             tc.tile_pool(name="psB", bufs=2, space="PSUM") as pB:

            # ---- resident buffers ----------------------------------------
            wq_g = [pp.tile([128, 4, HPC * HD], F16, tag=f"wq{g}",
                            name=f"wq{g}") for g in range(4)]
            wk_g = [pp.tile([128, 4, HD], F16, tag=f"wk{g}",
                            name=f"wk{g}") for g in range(4)]
            wv_g = [pp.tile([128, 4, HD], F16, tag=f"wv{g}",
                            name=f"wv{g}") for g in range(4)]
            wo_t = pp.tile([128, HPC, HID], F16, tag="wo")
            cosq_t = pp.tile([HD, NSTRIP, 2 * SW], F16, tag="cosq")
            sinq_t = pp.tile([HD, NSTRIP, 2 * SW], F16, tag="sinq")
            cosk_t = pp.tile([HD, S], F16, tag="cosk")
            sink_t = pp.tile([HD, S], F16, tag="sink")
            wqn_t = pp.tile([HD, 1], F32, tag="wqn")
            wkn_t = pp.tile([HD, 1], F32, tag="wkn")
            eps_t = pp.tile([HD, 1], F32, tag="eps")
            tm_t = pp.tile([128, 128], F16, tag="tm")
            sel_t = pp.tile([128, 4, SW], F16, tag="sel")
            ones_t = pp.tile([128, 128], F16, tag="ones")
            id_t = pp.tile([128, 128], F16, tag="ident")

            qt_all = pp.tile([HD, NSTRIP, 2 * SW], F16, tag="qt")
            kt_sb = pp.tile([HD, S], F16, tag="ktb")
            v_sb = pp.tile([128, S], F16, tag="vsb")
            ot_all = pp.tile([HD, NSTRIP, 2 * SW], F16, tag="ot")

            # PE warm-up: the HAM clock gate needs ~3.4us of sustained PE
            # activity to unthrottle (1.2 -> 2.4 GHz). Burn dummy matmuls on
            # a memset tile (no DMA dependency) while the first weight /
            # activation DMAs are in flight, so phase A starts at full clock.
            dmy = pp.tile([128, 256], F16, tag="dmy")
            nc.vector.memset(dmy[:], 0.0)
            dps = pA.tile([128, 256], F32, tag="big", name="warm")
            for _ in range(30):
                nc.tensor.matmul(dps[:], dmy[:, 0:128], dmy[:],
                                 start=True, stop=True)

            # ---- DMAs: interleave weight/activation chunks so the first
            # projection matmuls start after <1MB of traffic ----------------
            xt_s = [[None] * 4 for _ in range(NSTRIP)]

            def load_xt_chunk(s, g, eng=None):
                t = xp.tile([128, 4, SW], F16, tag=f"xt{g}", name=f"xt{s}_{g}")
                xt_s[s][g] = t
                base = (s * NHT + 4 * g) * SW
                (eng or nc.sync).dma_start(
                    t[:],
                    xT[:, base:base + 4 * SW].rearrange(
                        "p (a s) -> p a s", a=4))

            # spread the startup-critical issues across three DGE queues so
            # descriptor generation (~1us per dma_start) runs in parallel
            for g in range(4):
                nc.sync.dma_start(
                    wq_g[g][:],
                    wq[:, g * 4 * HPC * HD:(g + 1) * 4 * HPC * HD].rearrange(
                        "p (a d) -> p a d", a=4))
                nc.scalar.dma_start(
                    wk_g[g][:],
                    wk[:, g * 4 * HD:(g + 1) * 4 * HD].rearrange(
                        "p (a d) -> p a d", a=4))
                nc.scalar.dma_start(
                    wv_g[g][:],
                    wv[:, g * 4 * HD:(g + 1) * 4 * HD].rearrange(
                        "p (a d) -> p a d", a=4))
                load_xt_chunk(0, g, nc.gpsimd)
            for t, d in ((wqn_t, wqn), (wkn_t, wkn), (eps_t, epsb),
                         (tm_t, tmask), (ones_t, onesm), (id_t, ident)):
                nc.sync.dma_start(t[:], d[:])
            nc.sync.dma_start(sel_t[:],
                              selm[:].rearrange("p (a s) -> p a s", a=4))
            for g in range(4):
                load_xt_chunk(1, g)
            nc.sync.dma_start(cosq_t[:],
                              cosq[:].rearrange("p (a s) -> p a s", a=NSTRIP))
            nc.sync.dma_start(sinq_t[:],
                              sinq[:].rearrange("p (a s) -> p a s", a=NSTRIP))
            nc.sync.dma_start(cosk_t[:], cosk[:])
            nc.sync.dma_start(sink_t[:], sink[:])
            nc.sync.dma_start(wo_t[:], wo[:].rearrange("p (a d) -> p a d", a=HPC))

            def load_xt(s):
                for g in range(4):
                    load_xt_chunk(s, g)

            # ---- phase A: projections for one strip ----------------------
            def phase_a(s):
                _mark(nc, f"A{s}")
                q01 = pA.tile([128, 2 * SW], F32, tag="big", name=f"q01_{s}")
                kv = pA.tile([128, 2 * SW], F32, tag="big", name=f"kv_{s}")
                for ht in range(NHT):
                    g, j = ht // 4, ht % 4
                    xs = xt_s[s][g][:, j, :]
                    st_, sp_ = (ht == 0), (ht == NHT - 1)
                    nc.tensor.matmul(q01[:, 0:SW], wq_g[g][:, j, 0:HD],
                                     xs, start=st_, stop=sp_)
                    nc.tensor.matmul(q01[:, SW:2 * SW], wq_g[g][:, j, HD:2 * HD],
                                     xs, start=st_, stop=sp_)
                    nc.tensor.matmul(kv[:, 0:SW], wk_g[g][:, j, :], xs,
                                     start=st_, stop=sp_)
                    nc.tensor.matmul(kv[:, SW:2 * SW], wv_g[g][:, j, :], xs,
                                     start=st_, stop=sp_)
                return q01, kv

            # ---- epilogue: RMSNorm + RoPE for one strip ------------------
            def rope(dst, src, cos_ap, sin_ap, w):
                t1 = ep.tile([128, w], F16, tag=f"t1_{w}")
                u = ep.tile([128, w], F16, tag=f"u_{w}")
                nc.vector.tensor_tensor(out=t1[:], in0=src[:], in1=cos_ap,
                                        op=_ALU.mult)
                nc.vector.tensor_tensor(out=u[0:64, :], in0=src[64:128, :],
                                        in1=sin_ap[64:128], op=_ALU.mult)
                nc.vector.tensor_tensor(out=u[64:128, :], in0=src[0:64, :],
                                        in1=sin_ap[0:64], op=_ALU.mult)
                nc.vector.tensor_tensor(out=dst, in0=t1[:], in1=u[:],
                                        op=_ALU.add)

            def phase_epi(s, q01, kv):
                _mark(nc, f"epi{s}")
                sl = bass.ts(s, SW)
                qc = ep.tile([128, 2 * SW], F16, tag="qc")
                kc = ep.tile([128, SW], F16, tag="kc")
                vc = ep.tile([128, SW], F16, tag="vc")
                nc.vector.tensor_copy(qc[:], q01[:])
                nc.scalar.copy(kc[:], kv[:, 0:SW])
                nc.scalar.copy(vc[:], kv[:, SW:2 * SW])
                # V^T into v_sb via PE transposes
                for j in range(4):
                    tr = pB.tile([128, 128], F16, tag="b2", name=f"tr{s}_{j}")
                    nc.tensor.transpose(tr[:], vc[:, bass.ts(j, 128)], id_t[:])
                    nc.vector.tensor_copy(v_sb[:, bass.ts(4 * s + j, 128)], tr[:])
                # sum of squares -> rstd = exp(-0.5 * ln(ssq/HD + eps))
                sqq = ep.tile([128, 2 * SW], F16, tag="sqq")
                sqk = ep.tile([128, SW], F16, tag="sqk")
                nc.vector.tensor_tensor(out=sqq[:], in0=qc[:], in1=qc[:],
                                        op=_ALU.mult)
                nc.vector.tensor_tensor(out=sqk[:], in0=kc[:], in1=kc[:],
                                        op=_ALU.mult)
                ssq_q = pB.tile([128, 2 * SW], F32, tag="b2", name=f"ssqq{s}")
                ssq_k = pB.tile([128, SW], F32, tag="b2", name=f"ssqk{s}")
                nc.tensor.matmul(ssq_q[:, 0:SW], ones_t[:], sqq[:, 0:SW],
                                 start=True, stop=True)
                nc.tensor.matmul(ssq_q[:, SW:2 * SW], ones_t[:], sqq[:, SW:2 * SW],
                                 start=True, stop=True)
                nc.tensor.matmul(ssq_k[:], ones_t[:], sqk[:], start=True, stop=True)
                lnq = ep.tile([128, 2 * SW], F32, tag="lnq")
                lnk = ep.tile([128, SW], F32, tag="lnk")
                nc.scalar.activation(lnq[:], ssq_q[:], _ACT.Ln,
                                     scale=1.0 / HD, bias=eps_t[:])
                nc.scalar.activation(lnk[:], ssq_k[:], _ACT.Ln,
                                     scale=1.0 / HD, bias=eps_t[:])
                rsq = ep.tile([128, 2 * SW], F16, tag="rsq")
                rsk = ep.tile([128, SW], F16, tag="rsk")
                nc.scalar.activation(rsq[:], lnq[:], _ACT.Exp, scale=-0.5)
                nc.scalar.activation(rsk[:], lnk[:], _ACT.Exp, scale=-0.5)
                qn = ep.tile([128, 2 * SW], F16, tag="qn")
                kn = ep.tile([128, SW], F16, tag="kn")
                nc.vector.scalar_tensor_tensor(
                    out=qn[:], in0=qc[:], scalar=wqn_t[:], in1=rsq[:],
                    op0=_ALU.mult, op1=_ALU.mult)
                nc.vector.scalar_tensor_tensor(
                    out=kn[:], in0=kc[:], scalar=wkn_t[:], in1=rsk[:],
                    op0=_ALU.mult, op1=_ALU.mult)
                rope(qt_all[:, s, :], qn, cosq_t[:, s, :], sinq_t[:, s, :],
                     2 * SW)
                rope(kt_sb[:, sl], kn, cosk_t[:, sl], sink_t[:, sl], SW)

            # ---- phase B: attention for one strip (both heads fused).
            # cwork: list of closures, each emitting one out-projection unit
            # (4 matmuls + copies) -- interleaved as PE filler so the PE
            # never idles waiting on the exp chain (keeps HAM un-throttled).
            def phase_b(s, cwork=()):
                _mark(nc, f"B{s}")
                nk = 4 * s + 4
                cq = list(cwork)
                pv2 = pB.tile([128, 2 * SW], F32, tag="b2", name=f"pv{s}")
                dacc = ep.tile([128, 2 * SW], F16, tag="dacc")

                def emit_st(kt):
                    st = pA.tile([128, 2 * SW], F32, tag="big", name=f"st{s}_{kt}")
                    ks = kt_sb[:, bass.ts(kt, 128)]
                    off = kt - 4 * s
                    diag = off >= 0
                    for h in range(2):
                        hs = h * SW
                        nc.tensor.matmul(st[:, hs:hs + SW], ks,
                                         qt_all[:, s, hs:hs + SW],
                                         start=True, stop=not diag)
                        if diag:
                            # additive causal mask via PE: tmask encodes the
                            # triangle (rows 0-126) and full-mask (row 127);
                            # sel_t[off] routes each column to its mask row.
                            nc.tensor.matmul(st[:, hs:hs + SW], tm_t[:],
                                             sel_t[:, off, :],
                                             start=False, stop=True)
                    ex = xep.tile([128, 2 * SW], F16, tag="ex")
                    nc.scalar.activation(ex[:], st[:], _ACT.Exp, scale=ISQ)
                    return ex

                def emit_pvden(ex, kt):
                    st_, sp_ = (kt == 0), (kt == nk - 1)
                    vs_ = v_sb[:, bass.ts(kt, 128)]
                    nc.tensor.matmul(pv2[:, 0:SW], vs_, ex[:, 0:SW],
                                     start=st_, stop=sp_)
                    nc.tensor.matmul(pv2[:, SW:2 * SW], vs_, ex[:, SW:2 * SW],
                                     start=st_, stop=sp_)
                    # softmax denominator accumulates on the (idle) DVE
                    if kt == 0:
                        nc.vector.tensor_copy(dacc[:], ex[:])
                    else:
                        nc.vector.tensor_tensor(out=dacc[:], in0=dacc[:],
                                                in1=ex[:], op=_ALU.add)

                ncw = len(cq)
                ex_prev = emit_st(0)
                for kt in range(1, nk):
                    ex = emit_st(kt)
                    emit_pvden(ex_prev, kt - 1)
                    ex_prev = ex
                    # interleave out-projection filler, spread evenly
                    want = (kt * ncw) // (nk - 1) if nk > 1 else 0
                    while cq and (ncw - len(cq)) < want:
                        cq.pop(0)()
                emit_pvden(ex_prev, nk - 1)
                while cq:
                    cq.pop(0)()

                den2 = pB.tile([128, 2 * SW], F32, tag="b2", name=f"den{s}")
                nc.tensor.matmul(den2[:, 0:SW], ones_t[:], dacc[:, 0:SW],
                                 start=True, stop=True)
                nc.tensor.matmul(den2[:, SW:2 * SW], ones_t[:],
                                 dacc[:, SW:2 * SW], start=True, stop=True)
                rdl = ep.tile([128, 2 * SW], F32, tag="rdl")
                rden = ep.tile([128, 2 * SW], F32, tag="rden")
                nc.scalar.activation(rdl[:], den2[:], _ACT.Ln)
                nc.scalar.activation(rden[:], rdl[:], _ACT.Exp, scale=-1.0)
                nc.vector.tensor_tensor(out=ot_all[:, s, :], in0=pv2[:],
                                        in1=rden[:], op=_ALU.mult)

            # ---- phase C: output projection units ------------------------
            # Returns a list of 8 closures (one per [m-block, column-chunk]).
            # `alt_pool=True` (terminal strip) alternates ou between both
            # PSUM pools for a deeper copy pipeline; interleaved-into-B units
            # stay in pB only (pA is double-buffering the score tiles).
            def phase_c_units(s, alt_pool=False):
                _mark(nc, f"C{s}")
                obs = {}
                units = []

                def unit(mj, ch):
                    def emit():
                        m = 4 * s + mj
                        if mj not in obs:
                            obs[mj] = obp.tile([128, HID], F16, tag="ob",
                                               name=f"ob{m}")
                        ob = obs[mj]
                        pool = pA if (alt_pool and (2 * mj + ch) % 2) else pB
                        tg = "big" if pool is pA else "b2"
                        ou = pool.tile([128, 2 * SW], F32, tag=tg,
                                       name=f"ou{m}_{ch}")
                        for hf in range(2):
                            col = 1024 * ch + 512 * hf
                            nc.tensor.matmul(
                                ou[:, bass.ts(hf, SW)],
                                ot_all[:, s, bass.ts(mj, 128)],
                                wo_t[:, 0, col:col + SW], start=True, stop=False)
                            nc.tensor.matmul(
                                ou[:, bass.ts(hf, SW)],
                                ot_all[:, s, SW + 128 * mj:SW + 128 * mj + 128],
                                wo_t[:, 1, col:col + SW], start=False, stop=True)
                        base = 1024 * ch
                        # interleaved-into-B units copy on DVE only: the ACT
                        # FIFO is saturated with exps there, and a copy queued
                        # behind them stalls the ou slot rotation (PE filler)
                        if alt_pool and (2 * mj + ch) % 2:
                            nc.scalar.copy(ob[:, base:base + 2 * SW], ou[:])
                        else:
                            nc.vector.tensor_copy(ob[:, base:base + 2 * SW],
                                                  ou[:])
                        if ch == 1:
                            nc.gpsimd.dma_start(out[bass.ts(m, 128), :], ob[:])
                    return emit

                for mj in range(4):
                    for ch in range(2):
                        units.append(unit(mj, ch))
                return units

            # ---- schedule -------------------------------------------------
            a0 = phase_a(0)
            phase_epi(0, *a0)
            a1 = phase_a(1)
            phase_epi(1, *a1)
            load_xt(2)
            phase_b(0)
            a2 = phase_a(2)
            phase_epi(2, *a2)
            load_xt(3)
            phase_b(1, phase_c_units(0))
            a3 = phase_a(3)
            phase_epi(3, *a3)
            phase_b(2, phase_c_units(1))
            phase_b(3, phase_c_units(2))
            for u in phase_c_units(3, alt_pool=True):
                u()

    if legalize:
        legalize_waits(nc)
    return nc


# ---------------------------------------------------------------------------
# Host-side input prep.
def _rope_tables(position_ids: np.ndarray):
    pos = position_ids.reshape(-1).astype(np.float64)  # [S]
    j = np.arange(0, HD, 2, dtype=np.float64)
    inv_freq = 1.0 / (THETA ** (j / HD))               # [HD/2]
    freqs = np.outer(inv_freq, pos)                    # [HD/2, S]
    cos_h = np.cos(freqs)
    sin_h = np.sin(freqs)
    cosT = np.concatenate([cos_h, cos_h], axis=0)      # [HD, S]
    sinN = np.concatenate([sin_h, -sin_h], axis=0)
    return cosT, sinN


def _prep_in_maps(hidden_states, Wq, Wk, Wv, Wo, q_norm_w, k_norm_w,
                  position_ids):
    X = np.asarray(hidden_states, dtype=np.float32).reshape(S, HID)
    # [p, s, ht, c] <- X[512 s + c, 128 ht + p]
    xT = np.ascontiguousarray(
        X.reshape(NSTRIP, SW, NHT, 128).transpose(3, 0, 2, 1).reshape(
            128, NSTRIP * NHT * SW).astype(np.float16))
    cosT, sinN = _rope_tables(np.asarray(position_ids))
    # duplicated per-strip tables for the fused [q0|q1] layout
    cosq = np.concatenate(
        [np.concatenate([cosT[:, s * SW:(s + 1) * SW]] * 2, axis=1)
         for s in range(NSTRIP)], axis=1).astype(np.float16)
    sinq = np.concatenate(
        [np.concatenate([sinN[:, s * SW:(s + 1) * SW]] * 2, axis=1)
         for s in range(NSTRIP)], axis=1).astype(np.float16)
    cosk = np.ascontiguousarray(cosT.astype(np.float16))
    sink = np.ascontiguousarray(sinN.astype(np.float16))
    wqn = np.ascontiguousarray(
        np.asarray(q_norm_w, dtype=np.float32).reshape(HD, 1))
    wkn = np.ascontiguousarray(
        np.asarray(k_norm_w, dtype=np.float32).reshape(HD, 1))
    # Additive causal mask as a matmul: tmask[d, p] = MASKNEG where the
    # column routed to row d must be masked for query-partition... er,
    # st[p=kpos, col]: masked iff col_rel < p. tmask rows 0..126 carry the
    # triangle; row 127 is the all-masked row used for columns left of the
    # diagonal block. selm[:, off*SW + col] routes column col to its row.
    dd, pp_ = np.meshgrid(np.arange(128), np.arange(128), indexing="ij")
    tmask = np.where(dd < pp_, MASKNEG, 0.0)
    tmask[127, :] = MASKNEG
    tmask = tmask.astype(np.float16)
    selm = np.zeros((128, 4 * SW), np.float16)
    for off in range(4):
        vs = 128 * off
        base = off * SW
        for j in range(127):
            selm[j, base + vs + j] = 1.0
        selm[127, base:base + vs] = 1.0
    onesm = np.ones((128, 128), np.float16)
    ident = np.eye(128, dtype=np.float16)

    Wq = np.asarray(Wq, dtype=np.float32)
    Wk = np.asarray(Wk, dtype=np.float32)
    Wv = np.asarray(Wv, dtype=np.float32)
    Wo = np.asarray(Wo, dtype=np.float32)

    in_maps = []
    for c in range(NCORES):
        kv = c // (NCORES // NKV)
        wq_c = Wq[:, c * HPC * HD:(c + 1) * HPC * HD]
        wq_l = np.ascontiguousarray(
            wq_c.reshape(NHT, 128, HPC * HD).transpose(1, 0, 2).reshape(
                128, NHT * HPC * HD).astype(np.float16))
        wk_c = Wk[:, kv * HD:(kv + 1) * HD]
        wk_l = np.ascontiguousarray(
            wk_c.reshape(NHT, 128, HD).transpose(1, 0, 2).reshape(
                128, NHT * HD).astype(np.float16))
        wv_c = Wv[:, kv * HD:(kv + 1) * HD]
        wv_l = np.ascontiguousarray(
            wv_c.reshape(NHT, 128, HD).transpose(1, 0, 2).reshape(
                128, NHT * HD).astype(np.float16))
        wo_c = Wo[c * HPC * HD:(c + 1) * HPC * HD, :]
        wo_l = np.ascontiguousarray(
            wo_c.reshape(HPC, HD, HID).transpose(1, 0, 2).reshape(
                128, HPC * HID).astype(np.float16))
        in_maps.append({
            "xT": xT, "wq": wq_l, "wk": wk_l, "wv": wv_l, "wo": wo_l,
            "cosq": cosq, "sinq": sinq, "cosk": cosk, "sink": sink,
            "wqn": wqn, "wkn": wkn,
            "tmask": tmask, "selm": selm, "onesm": onesm, "ident": ident,
            "epsb": np.full((HD, 1), EPS, np.float32),
        })
    return in_maps


# ---------------------------------------------------------------------------
# Runner: persistent jitted shard_map over 8 cores (no donation so device
# buffers are reusable across timing iterations).
_CACHE: dict = {}


def _make_runner(nc):
    import jax
    from jax.sharding import Mesh, PartitionSpec
    try:
        from jax.experimental.shard_map import shard_map
    except ImportError:
        from jax.shard_map import shard_map
    from concourse.bass2jax import (_bass_exec_p, install_neuronx_cc_hook,
                                    partition_id_tensor)

    install_neuronx_cc_hook()

    partition_name = (nc.partition_id_tensor.name
                      if nc.partition_id_tensor else None)
    in_names, out_names, out_avals, zero_outs = [], [], [], []
    for alloc in nc.m.functions[0].allocations:
        if not isinstance(alloc, mybir.MemoryLocationSet):
            continue
        name = alloc.memorylocations[0].name
        if alloc.kind == "ExternalInput":
            if name != partition_name:
                in_names.append(name)
        elif alloc.kind == "ExternalOutput":
            shape = list(alloc.tensor_shape)
            npdt = mybir.dt.np(alloc.dtype)
            out_names.append(name)
            out_avals.append(jax.core.ShapedArray(shape, npdt))
            zero_outs.append(np.zeros(shape, npdt))

    n_params = len(in_names)
    all_in_names = list(in_names) + list(out_names)
    if partition_name is not None:
        all_in_names.append(partition_name)

    def _body(*args):
        operands = list(args)
        if partition_name is not None:
            operands.append(partition_id_tensor())
        outs = _bass_exec_p.bind(
            *operands,
            out_avals=tuple(out_avals),
            in_names=tuple(all_in_names),
            out_names=tuple(out_names),
            lowering_input_output_aliases=(),
            sim_require_finite=True,
            sim_require_nnan=True,
            nc=nc,
        )
        return tuple(outs)

    devices = jax.devices()[:NCORES]
    mesh = Mesh(np.asarray(devices), ("core",))
    n_outs = len(out_names)
    sharded = jax.jit(
        shard_map(_body, mesh=mesh,
                  in_specs=(PartitionSpec("core"),) * (n_params + n_outs),
                  out_specs=(PartitionSpec("core"),) * n_outs,
                  check_rep=False),
        keep_unused=True,
    )
    return {
        "fn": sharded, "in_names": in_names, "out_names": out_names,
        "out_avals": out_avals, "zero_outs": zero_outs, "jax": jax,
    }


def _get_runner(which="main"):
    key = f"runner_{which}"
    if key not in _CACHE:
        nc = build_nc() if which == "main" else build_null_nc()
        _CACHE[key] = _make_runner(nc)
    return _CACHE[key]


def _device_args(in_maps, which="main"):
    r = _get_runner(which)
    jax = r["jax"]
    concat_in = [
        np.concatenate([np.asarray(in_maps[c][name]) for c in range(NCORES)],
                       axis=0)
        for name in r["in_names"]
    ]
    concat_zeros = [
        np.zeros((NCORES * z.shape[0], *z.shape[1:]), z.dtype)
        for z in r["zero_outs"]
    ]
    return [jax.device_put(a) for a in (concat_in + concat_zeros)]


def _run(dargs, which="main"):
    r = _get_runner(which)
    outs = r["fn"](*dargs)
    return outs


def kernel(**inputs) -> np.ndarray:
    in_maps = _prep_in_maps(**inputs)
    dargs = _device_args(in_maps)
    outs = _run(dargs)
    out_c = np.asarray(outs[0]).reshape(NCORES, S, HID)
    full = out_c.sum(axis=0, dtype=np.float64).astype(np.float32)
    return full.reshape(B, S, HID)


def build_null_nc(legalize=True):
    """Input-identical null kernel: same ExternalInput/Output set, but only a
    trivial copy. Used to calibrate away per-dispatch input-staging overhead
    when estimating device execution time."""
    nc = bass.Bass()
    tensors = [
        ("xT", [128, NSTRIP * NHT * SW], F16),
        ("wq", [128, NHT * HPC * HD], F16),
        ("wk", [128, NHT * HD], F16), ("wv", [128, NHT * HD], F16),
        ("wo", [128, HPC * HID], F16),
        ("cosq", [HD, NSTRIP * 2 * SW], F16),
        ("sinq", [HD, NSTRIP * 2 * SW], F16),
        ("cosk", [HD, S], F16), ("sink", [HD, S], F16),
        ("wqn", [HD, 1], F32), ("wkn", [HD, 1], F32),
        ("tmask", [128, 128], F16), ("selm", [128, 4 * SW], F16),
        ("onesm", [128, 128], F16),
        ("ident", [128, 128], F16), ("epsb", [HD, 1], F32),
    ]
    handles = {}
    for name, shape, dt in tensors:
        handles[name] = nc.dram_tensor(name, shape, dt, kind="ExternalInput")
    out = nc.dram_tensor("out", [S, HID], F16, kind="ExternalOutput")
    with tile.TileContext(nc) as tc:
        with tc.tile_pool(name="sb", bufs=1) as sb:
            t = sb.tile([128, 128], F16)
            nc.sync.dma_start(t[:], handles["ident"][:])
            nc.sync.dma_start(out[0:128, 0:128], t[:])
    if legalize:
        legalize_waits(nc)
    return nc


def timed_run(inputs, iters=60):
    """Estimate on-device execution time via null-calibrated differencing
    (fallback when NTFF profiling is unavailable)."""
    import time
    in_maps = _prep_in_maps(**inputs)
    d_main = _device_args(in_maps, "main")
    d_null = _device_args(in_maps, "null")
    r_main = _get_runner("main")
    _get_runner("null")
    jax = r_main["jax"]
    jax.block_until_ready(_run(d_main, "main"))
    jax.block_until_ready(_run(d_null, "null"))

    tm, tn = [], []
    for _ in range(iters):
        t0 = time.perf_counter()
        jax.block_until_ready(_run(d_null, "null"))
        tn.append(time.perf_counter() - t0)
        t0 = time.perf_counter()
        jax.block_until_ready(_run(d_main, "main"))
        tm.append(time.perf_counter() - t0)
    tm, tn = np.array(tm), np.array(tn)
    est = float(np.median(tm) - np.median(tn))
    return max(est, 0.0), float(np.median(tm)), float(np.median(tn))


# revision 38
# speedup vs baseline: 1.0021x; 1.0021x over previous
"""Trainium2 Bass kernel for nn_Attention_53231824666818 (GQA attention block).

Sharding: tensor-parallel over heads across 8 NeuronCores. Core c owns query
heads {2c, 2c+1} and kv head c//4 (kv-head groups stay aligned to cores).
Each core computes a full-shape partial of the output projection (row-sharded
Wo); the host sums the 8 partials.

v2 redesign vs the first working kernel:
  * all operands staged in fp16 (halves HBM traffic, enables FWL weight
    loads and DVE 2x modes); PSUM accumulation stays fp32.
  * both query heads fused per tile: scores/exp/PV/softmax-denominator all
    operate on [128, 1024] tiles (head0 | head1), halving op counts.
  * every reciprocal/rsqrt computed as exp(-ln(x)) on the scalar engine --
    the DVE iterative-divide (3.3us per [128,512]) is gone, and ln/exp/copy/
    square all live in ONE activation table set (no table reloads).
  * causal mask applied additively on the score PSUM before a single full
    width exp (no subrange exp calls).
  * attention inner loop software-pipelined by one k-tile so the PE never
    waits on the exp.
  * weights/tables loaded in a handful of ~1MB DMAs; per-strip activations
    in one 2MB DMA; output staged fp16.
"""

import math

import numpy as np

import concourse.bass as bass
import concourse.tile as tile
from concourse import mybir

# ---------------------------------------------------------------------------
# Problem constants (hardcoded; kernel.py must be self-contained).
B, S, HID = 1, 2048, 2048
NH, NKV, HD = 16, 2, 128
EPS = 1e-6
THETA = 1000000.0
NCORES = 8
HPC = NH // NCORES          # query heads per core (2)
SW = 512                    # seq strip width
NSTRIP = S // SW            # 4
NHT = HID // 128            # hid-dim k-tiles (16)
ISQ = 1.0 / math.sqrt(HD)
MASKNEG = -60000.0

F32 = mybir.dt.float32
F16 = mybir.dt.float16

_ALU = mybir.AluOpType
_ACT = mybir.ActivationFunctionType


# ---------------------------------------------------------------------------
# Wait legalization: this walrus build caps fused sync waits at 1 per
# instruction (2 for event-semaphore ops) and rejects any wait on the
# LDWEIGHTS half of a lowered matmul. Tile can attach several waits to one
# instruction (notably the kernel-tail drain), so after TileContext exit we
# hoist excess waits onto same-engine InstNoOp's placed immediately before
# the owner, which blocks the sequencer identically.
_LW_COUNTER = [0]


def _wait_cap(ins) -> int:
    nm = type(ins).__name__
    if nm == "InstMatmult":
        return 0
    if "EventSem" in nm:
        return 2
    return 1


def legalize_waits(nc):
    for fn in nc.m.functions:
        for bb in fn.blocks:
            out = []
            changed = False
            for ins in bb.instructions:
                si = ins.sync_info
                if si is not None:
                    waits = list(si.on_wait or [])
                    cap = _wait_cap(ins)
                    if len(waits) > cap:
                        changed = True
                        for w in waits[cap:]:
                            _LW_COUNTER[0] += 1
                            nop = mybir.InstNoOp(
                                name=f"I-lw-{_LW_COUNTER[0]}",
                                engine=ins.engine,
                                sync_info=mybir.SyncInfo(on_wait=[w], on_update=[]),
                            )
                            out.append(nop)
                        ins.sync_info = mybir.SyncInfo(
                            on_wait=waits[:cap], on_update=list(si.on_update or [])
                        )
                out.append(ins)
            if changed:
                bb.instructions = out
    return nc


# ---------------------------------------------------------------------------
PHASE_MARKS = []


def _mark(nc, label):
    PHASE_MARKS.append((label, int(nc.get_next_instruction_name().split("-")[1])))


def build_nc(legalize=True):
    PHASE_MARKS.clear()
    nc = bass.Bass()

    # host-pretiled activations: column (s*NHT + ht)*SW + c holds
    # X[512 s + c, 128 ht + p] -- every DMA chunk is contiguous per partition
    xT = nc.dram_tensor("xT", [128, NSTRIP * NHT * SW], F16,
                        kind="ExternalInput")
    wq = nc.dram_tensor("wq", [128, NHT * HPC * HD], F16, kind="ExternalInput")
    wk = nc.dram_tensor("wk", [128, NHT * HD], F16, kind="ExternalInput")
    wv = nc.dram_tensor("wv", [128, NHT * HD], F16, kind="ExternalInput")
    wo = nc.dram_tensor("wo", [128, HPC * HID], F16, kind="ExternalInput")
    cosq = nc.dram_tensor("cosq", [HD, NSTRIP * 2 * SW], F16, kind="ExternalInput")
    sinq = nc.dram_tensor("sinq", [HD, NSTRIP * 2 * SW], F16, kind="ExternalInput")
    cosk = nc.dram_tensor("cosk", [HD, S], F16, kind="ExternalInput")
    sink = nc.dram_tensor("sink", [HD, S], F16, kind="ExternalInput")
    wqn = nc.dram_tensor("wqn", [HD, 1], F32, kind="ExternalInput")
    wkn = nc.dram_tensor("wkn", [HD, 1], F32, kind="ExternalInput")
    tmask = nc.dram_tensor("tmask", [128, 128], F16, kind="ExternalInput")
    selm = nc.dram_tensor("selm", [128, 4 * 2 * SW], F16, kind="ExternalInput")
    onesm = nc.dram_tensor("onesm", [128, 128], F16, kind="ExternalInput")
    ident = nc.dram_tensor("ident", [128, 128], F16, kind="ExternalInput")
    epsb = nc.dram_tensor("epsb", [HD, 1], F32, kind="ExternalInput")
    out = nc.dram_tensor("out", [S, HID], F16, kind="ExternalOutput")

    with tile.TileContext(nc) as tc:
        with tc.tile_pool(name="persist", bufs=1) as pp, \
             tc.tile_pool(name="xtp", bufs=2) as xp, \
             tc.tile_pool(name="epi", bufs=2) as ep, \
             tc.tile_pool(name="exp", bufs=3) as xep, \
             tc.tile_pool(name="obp", bufs=2) as obp, \
             tc.tile_pool(name="psA", bufs=2, space="PSUM") as pA, \
             tc.tile_pool(name="psB", bufs=2, space="PSUM") as pB:

            # ---- resident buffers ----------------------------------------
            wq_g = [pp.tile([128, 4, HPC * HD], F16, tag=f"wq{g}",
                            name=f"wq{g}") for g in range(4)]
            wk_g = [pp.tile([128, 4, HD], F16, tag=f"wk{g}",
                            name=f"wk{g}") for g in range(4)]
            wv_g = [pp.tile([128, 4, HD], F16, tag=f"wv{g}",
                            name=f"wv{g}") for g in range(4)]
            wo_t = pp.tile([128, HPC, HID], F16, tag="wo")
            cosq_t = pp.tile([HD, NSTRIP, 2 * SW], F16, tag="cosq")
            sinq_t = pp.tile([HD, NSTRIP, 2 * SW], F16, tag="sinq")
            cosk_t = pp.tile([HD, S], F16, tag="cosk")
            sink_t = pp.tile([HD, S], F16, tag="sink")
            wqn_t = pp.tile([HD, 1], F32, tag="wqn")
            wkn_t = pp.tile([HD, 1], F32, tag="wkn")
            eps_t = pp.tile([HD, 1], F32, tag="eps")
            tm_t = pp.tile([128, 128], F16, tag="tm")
            sel_t = pp.tile([128, 4, 2 * SW], F16, tag="sel")
            ones_t = pp.tile([128, 128], F16, tag="ones")
            id_t = pp.tile([128, 128], F16, tag="ident")

            qt_all = pp.tile([HD, NSTRIP, 2 * SW], F16, tag="qt")
            kt_sb = pp.tile([HD, S], F16, tag="ktb")
            v_sb = pp.tile([128, S], F16, tag="vsb")
            ot_all = pp.tile([HD, NSTRIP, 2 * SW], F16, tag="ot")

            # PE warm-up: the HAM clock gate needs ~3.4us of sustained PE
            # activity to unthrottle (1.2 -> 2.4 GHz). Burn dummy matmuls on
            # a memset tile (no DMA dependency) while the first weight /
            # activation DMAs are in flight, so phase A starts at full clock.
            dmy = pp.tile([128, 256], F16, tag="dmy")
            nc.vector.memset(dmy[:], 0.0)
            dps = pA.tile([128, 256], F32, tag="big", name="warm")
            for _ in range(16):
                nc.tensor.matmul(dps[:], dmy[:, 0:128], dmy[:],
                                 start=True, stop=True)

            # ---- DMAs: interleave weight/activation chunks so the first
            # projection matmuls start after <1MB of traffic ----------------
            xt_s = [[None] * 4 for _ in range(NSTRIP)]

            def load_xt_chunk(s, g, eng=None):
                t = xp.tile([128, 4, SW], F16, tag=f"xt{g}", name=f"xt{s}_{g}")
                xt_s[s][g] = t
                base = (s * NHT + 4 * g) * SW
                (eng or nc.sync).dma_start(
                    t[:],
                    xT[:, base:base + 4 * SW].rearrange(
                        "p (a s) -> p a s", a=4))

            # spread the startup-critical issues across three DGE queues so
            # descriptor generation (~1us per dma_start) runs in parallel
            for g in range(4):
                nc.sync.dma_start(
                    wq_g[g][:],
                    wq[:, g * 4 * HPC * HD:(g + 1) * 4 * HPC * HD].rearrange(
                        "p (a d) -> p a d", a=4))
                nc.scalar.dma_start(
                    wk_g[g][:],
                    wk[:, g * 4 * HD:(g + 1) * 4 * HD].rearrange(
                        "p (a d) -> p a d", a=4))
                nc.scalar.dma_start(
                    wv_g[g][:],
                    wv[:, g * 4 * HD:(g + 1) * 4 * HD].rearrange(
                        "p (a d) -> p a d", a=4))
                load_xt_chunk(0, g, nc.gpsimd)
            for t, d in ((wqn_t, wqn), (wkn_t, wkn), (eps_t, epsb),
                         (tm_t, tmask), (ones_t, onesm), (id_t, ident)):
                nc.sync.dma_start(t[:], d[:])
            nc.sync.dma_start(sel_t[:],
                              selm[:].rearrange("p (a s) -> p a s", a=4))
            for g in range(4):
                load_xt_chunk(1, g)
            nc.sync.dma_start(cosq_t[:],
                              cosq[:].rearrange("p (a s) -> p a s", a=NSTRIP))
            nc.sync.dma_start(sinq_t[:],
                              sinq[:].rearrange("p (a s) -> p a s", a=NSTRIP))
            nc.sync.dma_start(cosk_t[:], cosk[:])
            nc.sync.dma_start(sink_t[:], sink[:])
            nc.sync.dma_start(wo_t[:], wo[:].rearrange("p (a d) -> p a d", a=HPC))

            def load_xt(s):
                for g in range(4):
                    load_xt_chunk(s, g)

            # ---- phase A: projections for one strip ----------------------
            def phase_a(s):
                _mark(nc, f"A{s}")
                q01 = pA.tile([128, 2 * SW], F32, tag="big", name=f"q01_{s}")
                kv = pA.tile([128, 2 * SW], F32, tag="big", name=f"kv_{s}")
                for ht in range(NHT):
                    g, j = ht // 4, ht % 4
                    xs = xt_s[s][g][:, j, :]
                    st_, sp_ = (ht == 0), (ht == NHT - 1)
                    nc.tensor.matmul(q01[:, 0:SW], wq_g[g][:, j, 0:HD],
                                     xs, start=st_, stop=sp_)
                    nc.tensor.matmul(q01[:, SW:2 * SW], wq_g[g][:, j, HD:2 * HD],
                                     xs, start=st_, stop=sp_)
                    nc.tensor.matmul(kv[:, 0:SW], wk_g[g][:, j, :], xs,
                                     start=st_, stop=sp_)
                    nc.tensor.matmul(kv[:, SW:2 * SW], wv_g[g][:, j, :], xs,
                                     start=st_, stop=sp_)
                return q01, kv

            # ---- epilogue: RMSNorm + RoPE for one strip ------------------
            def rope(dst, src, cos_ap, sin_ap, w):
                t1 = ep.tile([128, w], F16, tag=f"t1_{w}")
                u = ep.tile([128, w], F16, tag=f"u_{w}")
                nc.vector.tensor_tensor(out=t1[:], in0=src[:], in1=cos_ap,
                                        op=_ALU.mult)
                nc.vector.tensor_tensor(out=u[0:64, :], in0=src[64:128, :],
                                        in1=sin_ap[64:128], op=_ALU.mult)
                nc.vector.tensor_tensor(out=u[64:128, :], in0=src[0:64, :],
                                        in1=sin_ap[0:64], op=_ALU.mult)
                nc.vector.tensor_tensor(out=dst, in0=t1[:], in1=u[:],
                                        op=_ALU.add)

            def phase_epi(s, q01, kv):
                _mark(nc, f"epi{s}")
                sl = bass.ts(s, SW)
                qc = ep.tile([128, 2 * SW], F16, tag="qc")
                kc = ep.tile([128, SW], F16, tag="kc")
                vc = ep.tile([128, SW], F16, tag="vc")
                nc.vector.tensor_copy(qc[:], q01[:])
                nc.scalar.copy(kc[:], kv[:, 0:SW])
                nc.scalar.copy(vc[:], kv[:, SW:2 * SW])
                # V^T into v_sb via PE transposes
                for j in range(4):
                    tr = pB.tile([128, 128], F16, tag="b2", name=f"tr{s}_{j}")
                    nc.tensor.transpose(tr[:], vc[:, bass.ts(j, 128)], id_t[:])
                    nc.vector.tensor_copy(v_sb[:, bass.ts(4 * s + j, 128)], tr[:])
                # sum of squares -> rstd = exp(-0.5 * ln(ssq/HD + eps))
                sqq = ep.tile([128, 2 * SW], F16, tag="sqq")
                sqk = ep.tile([128, SW], F16, tag="sqk")
                nc.vector.tensor_tensor(out=sqq[:], in0=qc[:], in1=qc[:],
                                        op=_ALU.mult)
                nc.vector.tensor_tensor(out=sqk[:], in0=kc[:], in1=kc[:],
                                        op=_ALU.mult)
                ssq_q = pB.tile([128, 2 * SW], F32, tag="b2", name=f"ssqq{s}")
                ssq_k = pB.tile([128, SW], F32, tag="b2", name=f"ssqk{s}")
                nc.tensor.matmul(ssq_q[:, 0:SW], ones_t[:], sqq[:, 0:SW],
                                 start=True, stop=True)
                nc.tensor.matmul(ssq_q[:, SW:2 * SW], ones_t[:], sqq[:, SW:2 * SW],
                                 start=True, stop=True)
                nc.tensor.matmul(ssq_k[:], ones_t[:], sqk[:], start=True, stop=True)
                lnq = ep.tile([128, 2 * SW], F32, tag="lnq")
                lnk = ep.tile([128, SW], F32, tag="lnk")
                nc.scalar.activation(lnq[:], ssq_q[:], _ACT.Ln,
                                     scale=1.0 / HD, bias=eps_t[:])
                nc.scalar.activation(lnk[:], ssq_k[:], _ACT.Ln,
                                     scale=1.0 / HD, bias=eps_t[:])
                rsq = ep.tile([128, 2 * SW], F16, tag="rsq")
                rsk = ep.tile([128, SW], F16, tag="rsk")
                nc.scalar.activation(rsq[:], lnq[:], _ACT.Exp, scale=-0.5)
                nc.scalar.activation(rsk[:], lnk[:], _ACT.Exp, scale=-0.5)
                qn = ep.tile([128, 2 * SW], F16, tag="qn")
                kn = ep.tile([128, SW], F16, tag="kn")
                nc.vector.scalar_tensor_tensor(
                    out=qn[:], in0=qc[:], scalar=wqn_t[:], in1=rsq[:],
                    op0=_ALU.mult, op1=_ALU.mult)
                nc.vector.scalar_tensor_tensor(
                    out=kn[:], in0=kc[:], scalar=wkn_t[:], in1=rsk[:],
                    op0=_ALU.mult, op1=_ALU.mult)
                rope(qt_all[:, s, :], qn, cosq_t[:, s, :], sinq_t[:, s, :],
                     2 * SW)
                rope(kt_sb[:, sl], kn, cosk_t[:, sl], sink_t[:, sl], SW)

            # ---- phase B: attention for one strip (both heads fused).
            # cwork: list of closures, each emitting one out-projection unit
            # (4 matmuls + copies) -- interleaved as PE filler so the PE
            # never idles waiting on the exp chain (keeps HAM un-throttled).
            def phase_b(s, cwork=()):
                _mark(nc, f"B{s}")
                nk = 4 * s + 4
                cq = list(cwork)
                pv2 = pB.tile([128, 2 * SW], F32, tag="b2", name=f"pv{s}")
                dacc = ep.tile([128, 2 * SW], F16, tag="dacc")

                def emit_st(kt):
                    st = pA.tile([128, 2 * SW], F32, tag="big", name=f"st{s}_{kt}")
                    ks = kt_sb[:, bass.ts(kt, 128)]
                    off = kt - 4 * s
                    diag = off >= 0
                    for h in range(2):
                        hs = h * SW
                        nc.tensor.matmul(st[:, hs:hs + SW], ks,
                                         qt_all[:, s, hs:hs + SW],
                                         start=True, stop=not diag)
                        if diag:
                            nc.tensor.matmul(st[:, hs:hs + SW], tm_t[:],
                                             sel_t[:, off, h * SW:(h + 1) * SW],
                                             start=False, stop=True)
                    ex = xep.tile([128, 2 * SW], F16, tag="ex")
                    nc.scalar.activation(ex[:], st[:], _ACT.Exp, scale=ISQ)
                    return ex

                def emit_pvden(ex, kt):
                    st_, sp_ = (kt == 0), (kt == nk - 1)
                    vs_ = v_sb[:, bass.ts(kt, 128)]
                    nc.tensor.matmul(pv2[:, 0:SW], vs_, ex[:, 0:SW],
                                     start=st_, stop=sp_)
                    nc.tensor.matmul(pv2[:, SW:2 * SW], vs_, ex[:, SW:2 * SW],
                                     start=st_, stop=sp_)
                    # softmax denominator accumulates on the (idle) DVE
                    if kt == 0:
                        nc.vector.tensor_copy(dacc[:], ex[:])
                    else:
                        nc.vector.tensor_tensor(out=dacc[:], in0=dacc[:],
                                                in1=ex[:], op=_ALU.add)

                ncw = len(cq)
                ex_prev = emit_st(0)
                for kt in range(1, nk):
                    ex = emit_st(kt)
                    emit_pvden(ex_prev, kt - 1)
                    ex_prev = ex
                    # interleave out-projection filler, spread evenly
                    want = (kt * ncw) // (nk - 1) if nk > 1 else 0
                    while cq and (ncw - len(cq)) < want:
                        cq.pop(0)()
                emit_pvden(ex_prev, nk - 1)
                while cq:
                    cq.pop(0)()

                den2 = pB.tile([128, 2 * SW], F32, tag="b2", name=f"den{s}")
                nc.tensor.matmul(den2[:, 0:SW], ones_t[:], dacc[:, 0:SW],
                                 start=True, stop=True)
                nc.tensor.matmul(den2[:, SW:2 * SW], ones_t[:],
                                 dacc[:, SW:2 * SW], start=True, stop=True)
                rdl = ep.tile([128, 2 * SW], F32, tag="rdl")
                rden = ep.tile([128, 2 * SW], F32, tag="rden")
                nc.scalar.activation(rdl[:], den2[:], _ACT.Ln)
                nc.scalar.activation(rden[:], rdl[:], _ACT.Exp, scale=-1.0)
                nc.vector.tensor_tensor(out=ot_all[:, s, :], in0=pv2[:],
                                        in1=rden[:], op=_ALU.mult)

            # ---- phase C: output projection units ------------------------
            # Returns a list of 8 closures (one per [m-block, column-chunk]).
            # `alt_pool=True` (terminal strip) alternates ou between both
            # PSUM pools for a deeper copy pipeline; interleaved-into-B units
            # stay in pB only (pA is double-buffering the score tiles).
            def phase_c_units(s, alt_pool=False):
                _mark(nc, f"C{s}")
                obs = {}
                units = []

                def unit(mj, ch):
                    def emit():
                        m = 4 * s + mj
                        if mj not in obs:
                            obs[mj] = obp.tile([128, HID], F16, tag="ob",
                                               name=f"ob{m}")
                        ob = obs[mj]
                        pool = pA if (alt_pool and (2 * mj + ch) % 2) else pB
                        tg = "big" if pool is pA else "b2"
                        ou = pool.tile([128, 2 * SW], F32, tag=tg,
                                       name=f"ou{m}_{ch}")
                        for hf in range(2):
                            col = 1024 * ch + 512 * hf
                            nc.tensor.matmul(
                                ou[:, bass.ts(hf, SW)],
                                ot_all[:, s, bass.ts(mj, 128)],
                                wo_t[:, 0, col:col + SW], start=True, stop=False)
                            nc.tensor.matmul(
                                ou[:, bass.ts(hf, SW)],
                                ot_all[:, s, SW + 128 * mj:SW + 128 * mj + 128],
                                wo_t[:, 1, col:col + SW], start=False, stop=True)
                        base = 1024 * ch
                        if (2 * mj + ch) % 2:
                            nc.scalar.copy(ob[:, base:base + 2 * SW], ou[:])
                        else:
                            nc.vector.tensor_copy(ob[:, base:base + 2 * SW],
                                                  ou[:])
                        if ch == 1:
                            nc.gpsimd.dma_start(out[bass.ts(m, 128), :], ob[:])
                    return emit

                for mj in range(4):
                    for ch in range(2):
                        units.append(unit(mj, ch))
                return units

            # ---- schedule -------------------------------------------------
            a0 = phase_a(0)
            phase_epi(0, *a0)
            a1 = phase_a(1)
            phase_epi(1, *a1)
            load_xt(2)
            phase_b(0)
            a2 = phase_a(2)
            phase_epi(2, *a2)
            load_xt(3)
            phase_b(1, phase_c_units(0))
            a3 = phase_a(3)
            phase_epi(3, *a3)
            phase_b(2, phase_c_units(1))
            phase_b(3, phase_c_units(2))
            for u in phase_c_units(3, alt_pool=True):
                u()

    if legalize:
        legalize_waits(nc)
    return nc


# ---------------------------------------------------------------------------
# Host-side input prep.
def _rope_tables(position_ids: np.ndarray):
    pos = position_ids.reshape(-1).astype(np.float64)  # [S]
    j = np.arange(0, HD, 2, dtype=np.float64)
    inv_freq = 1.0 / (THETA ** (j / HD))               # [HD/2]
    freqs = np.outer(inv_freq, pos)                    # [HD/2, S]
    cos_h = np.cos(freqs)
    sin_h = np.sin(freqs)
    cosT = np.concatenate([cos_h, cos_h], axis=0)      # [HD, S]
    sinN = np.concatenate([sin_h, -sin_h], axis=0)
    return cosT, sinN


def _prep_in_maps(hidden_states, Wq, Wk, Wv, Wo, q_norm_w, k_norm_w,
                  position_ids):
    X = np.asarray(hidden_states, dtype=np.float32).reshape(S, HID)
    # [p, s, ht, c] <- X[512 s + c, 128 ht + p]
    xT = np.ascontiguousarray(
        X.reshape(NSTRIP, SW, NHT, 128).transpose(3, 0, 2, 1).reshape(
            128, NSTRIP * NHT * SW).astype(np.float16))
    cosT, sinN = _rope_tables(np.asarray(position_ids))
    # duplicated per-strip tables for the fused [q0|q1] layout
    cosq = np.concatenate(
        [np.concatenate([cosT[:, s * SW:(s + 1) * SW]] * 2, axis=1)
         for s in range(NSTRIP)], axis=1).astype(np.float16)
    sinq = np.concatenate(
        [np.concatenate([sinN[:, s * SW:(s + 1) * SW]] * 2, axis=1)
         for s in range(NSTRIP)], axis=1).astype(np.float16)
    cosk = np.ascontiguousarray(cosT.astype(np.float16))
    sink = np.ascontiguousarray(sinN.astype(np.float16))
    wqn = np.ascontiguousarray(
        np.asarray(q_norm_w, dtype=np.float32).reshape(HD, 1))
    wkn = np.ascontiguousarray(
        np.asarray(k_norm_w, dtype=np.float32).reshape(HD, 1))
    # Additive causal mask as a matmul: tmask[d, p] = MASKNEG where the
    # column routed to row d must be masked for query-partition... er,
    # st[p=kpos, col]: masked iff col_rel < p. tmask rows 0..126 carry the
    # triangle; row 127 is the all-masked row used for columns left of the
    # diagonal block. selm[:, off*SW + col] routes column col to its row.
    dd, pp_ = np.meshgrid(np.arange(128), np.arange(128), indexing="ij")
    tmask = np.where(dd < pp_, MASKNEG, 0.0)
    tmask[127, :] = MASKNEG
    tmask = tmask.astype(np.float16)
    selm = np.zeros((128, 4 * 2 * SW), np.float16)
    for off in range(4):
        vs = 128 * off
        for half in range(2):
            base = off * 2 * SW + half * SW
            for j in range(127):
                selm[j, base + vs + j] = 1.0
            selm[127, base:base + vs] = 1.0
    onesm = np.ones((128, 128), np.float16)
    ident = np.eye(128, dtype=np.float16)

    Wq = np.asarray(Wq, dtype=np.float32)
    Wk = np.asarray(Wk, dtype=np.float32)
    Wv = np.asarray(Wv, dtype=np.float32)
    Wo = np.asarray(Wo, dtype=np.float32)

    in_maps = []
    for c in range(NCORES):
        kv = c // (NCORES // NKV)
        wq_c = Wq[:, c * HPC * HD:(c + 1) * HPC * HD]
        wq_l = np.ascontiguousarray(
            wq_c.reshape(NHT, 128, HPC * HD).transpose(1, 0, 2).reshape(
                128, NHT * HPC * HD).astype(np.float16))
        wk_c = Wk[:, kv * HD:(kv + 1) * HD]
        wk_l = np.ascontiguousarray(
            wk_c.reshape(NHT, 128, HD).transpose(1, 0, 2).reshape(
                128, NHT * HD).astype(np.float16))
        wv_c = Wv[:, kv * HD:(kv + 1) * HD]
        wv_l = np.ascontiguousarray(
            wv_c.reshape(NHT, 128, HD).transpose(1, 0, 2).reshape(
                128, NHT * HD).astype(np.float16))
        wo_c = Wo[c * HPC * HD:(c + 1) * HPC * HD, :]
        wo_l = np.ascontiguousarray(
            wo_c.reshape(HPC, HD, HID).transpose(1, 0, 2).reshape(
                128, HPC * HID).astype(np.float16))
        in_maps.append({
            "xT": xT, "wq": wq_l, "wk": wk_l, "wv": wv_l, "wo": wo_l,
            "cosq": cosq, "sinq": sinq, "cosk": cosk, "sink": sink,
            "wqn": wqn, "wkn": wkn,
            "tmask": tmask, "selm": selm, "onesm": onesm, "ident": ident,
            "epsb": np.full((HD, 1), EPS, np.float32),
        })
    return in_maps


# ---------------------------------------------------------------------------
# Runner: persistent jitted shard_map over 8 cores (no donation so device
# buffers are reusable across timing iterations).
_CACHE: dict = {}


def _make_runner(nc):
    import jax
    from jax.sharding import Mesh, PartitionSpec
    try:
        from jax.experimental.shard_map import shard_map
    except ImportError:
        from jax.shard_map import shard_map
    from concourse.bass2jax import (_bass_exec_p, install_neuronx_cc_hook,
                                    partition_id_tensor)

    install_neuronx_cc_hook()

    partition_name = (nc.partition_id_tensor.name
                      if nc.partition_id_tensor else None)
    in_names, out_names, out_avals, zero_outs = [], [], [], []
    for alloc in nc.m.functions[0].allocations:
        if not isinstance(alloc, mybir.MemoryLocationSet):
            continue
        name = alloc.memorylocations[0].name
        if alloc.kind == "ExternalInput":
            if name != partition_name:
                in_names.append(name)
        elif alloc.kind == "ExternalOutput":
            shape = list(alloc.tensor_shape)
            npdt = mybir.dt.np(alloc.dtype)
            out_names.append(name)
            out_avals.append(jax.core.ShapedArray(shape, npdt))
            zero_outs.append(np.zeros(shape, npdt))

    n_params = len(in_names)
    all_in_names = list(in_names) + list(out_names)
    if partition_name is not None:
        all_in_names.append(partition_name)

    def _body(*args):
        operands = list(args)
        if partition_name is not None:
            operands.append(partition_id_tensor())
        outs = _bass_exec_p.bind(
            *operands,
            out_avals=tuple(out_avals),
            in_names=tuple(all_in_names),
            out_names=tuple(out_names),
            lowering_input_output_aliases=(),
            sim_require_finite=True,
            sim_require_nnan=True,
            nc=nc,
        )
        return tuple(outs)

    devices = jax.devices()[:NCORES]
    mesh = Mesh(np.asarray(devices), ("core",))
    n_outs = len(out_names)
    sharded = jax.jit(
        shard_map(_body, mesh=mesh,
                  in_specs=(PartitionSpec("core"),) * (n_params + n_outs),
                  out_specs=(PartitionSpec("core"),) * n_outs,
                  check_rep=False),
        keep_unused=True,
    )
    return {
        "fn": sharded, "in_names": in_names, "out_names": out_names,
        "out_avals": out_avals, "zero_outs": zero_outs, "jax": jax,
    }


def _get_runner(which="main"):
    key = f"runner_{which}"
    if key not in _CACHE:
        nc = build_nc() if which == "main" else build_null_nc()
        _CACHE[key] = _make_runner(nc)
    return _CACHE[key]


def _device_args(in_maps, which="main"):
    r = _get_runner(which)
    jax = r["jax"]
    concat_in = [
        np.concatenate([np.asarray(in_maps[c][name]) for c in range(NCORES)],
                       axis=0)
        for name in r["in_names"]
    ]
    concat_zeros = [
        np.zeros((NCORES * z.shape[0], *z.shape[1:]), z.dtype)
        for z in r["zero_outs"]
    ]
    return [jax.device_put(a) for a in (concat_in + concat_zeros)]


def _run(dargs, which="main"):
    r = _get_runner(which)
    outs = r["fn"](*dargs)
    return outs


def kernel(**inputs) -> np.ndarray:
    in_maps = _prep_in_maps(**inputs)
    dargs = _device_args(in_maps)
    outs = _run(dargs)
    out_c = np.asarray(outs[0]).reshape(NCORES, S, HID)
    full = out_c.sum(axis=0, dtype=np.float64).astype(np.float32)
    return full.reshape(B, S, HID)


def build_null_nc(legalize=True):
    """Input-identical null kernel: same ExternalInput/Output set, but only a
    trivial copy. Used to calibrate away per-dispatch input-staging overhead
    when estimating device execution time."""
    nc = bass.Bass()
    tensors = [
        ("xT", [128, NSTRIP * NHT * SW], F16),
        ("wq", [128, NHT * HPC * HD], F16),
        ("wk", [128, NHT * HD], F16), ("wv", [128, NHT * HD], F16),
        ("wo", [128, HPC * HID], F16),
        ("cosq", [HD, NSTRIP * 2 * SW], F16),
        ("sinq", [HD, NSTRIP * 2 * SW], F16),
        ("cosk", [HD, S], F16), ("sink", [HD, S], F16),
        ("wqn", [HD, 1], F32), ("wkn", [HD, 1], F32),
        ("tmask", [128, 128], F16), ("selm", [128, 4 * 2 * SW], F16),
        ("onesm", [128, 128], F16),
        ("ident", [128, 128], F16), ("epsb", [HD, 1], F32),
    ]
    handles = {}
    for name, shape, dt in tensors:
        handles[name] = nc.dram_tensor(name, shape, dt, kind="ExternalInput")
    out = nc.dram_tensor("out", [S, HID], F16, kind="ExternalOutput")
    with tile.TileContext(nc) as tc:
        with tc.tile_pool(name="sb", bufs=1) as sb:
            t = sb.tile([128, 128], F16)
            nc.sync.dma_start(t[:], handles["ident"][:])
            nc.sync.dma_start(out[0:128, 0:128], t[:])
    if legalize:
        legalize_waits(nc)
    return nc


def timed_run(inputs, iters=60):
    """Estimate on-device execution time via null-calibrated differencing
    (fallback when NTFF profiling is unavailable)."""
    import time
    in_maps = _prep_in_maps(**inputs)
    d_main = _device_args(in_maps, "main")
    d_null = _device_args(in_maps, "null")
    r_main = _get_runner("main")
    _get_runner("null")
    jax = r_main["jax"]
    jax.block_until_ready(_run(d_main, "main"))
    jax.block_until_ready(_run(d_null, "null"))

    tm, tn = [], []
    for _ in range(iters):
        t0 = time.perf_counter()
        jax.block_until_ready(_run(d_null, "null"))
        tn.append(time.perf_counter() - t0)
        t0 = time.perf_counter()
        jax.block_until_ready(_run(d_main, "main"))
        tm.append(time.perf_counter() - t0)
    tm, tn = np.array(tm), np.array(tn)
    est = float(np.median(tm) - np.median(tn))
    return max(est, 0.0), float(np.median(tm)), float(np.median(tn))
